# revision 14
# baseline (speedup 1.0000x reference)
"""Trainium2 Bass kernel for nn_Net_41223096107028.

Computes the 4-iteration argaug/attention/masked-MLP loss of reference.py
on 8 NeuronCores, data-parallel over the 2048 (b,t) rows (256 rows/core,
2 partition-tiles of 128).

Per iteration:
  - sliding correlation num[p,s] = <y_res[p], window_s(x_res[p])> via an
    exact 255-point circular DFT on the PE array: num = IDFT(F(x)conj(F(y)))
    with fixed real DFT matrices (255 = 2*128-1, so circular == linear
    correlation exactly; per-row correlations can't be a direct matmul, but
    the DFT factorization shares its matrices across rows). 8 fp32 matmuls
    per iteration over all 256 rows replaces 2040 truncated-window DVE
    reduce ops.
  - window norms via two cancellation-free DVE prefix scans of x^2,
  - argmax over num/sqrt(ss) (||y|| > 0 is a common positive factor and is
    dropped; reciprocal via the 1-instruction approx op),
  - per-row window gathers via indirect DMA on a DRAM mirror (per-partition
    offsets; gpsimd indirect_copy shares indices across 16-partition groups
    so it cannot do per-row shifts),
  - softmax folded into a second ACT Exp pass with bias = -max - ln(sum),
  - the 2-layer channel-masked MLP as 4 PE matmuls in transposed layout
    (only the active 256-channel slice is computed),
  - loss via ||y_res_new||^2 (y_ele - y_res = -y_res_new), accumulated
    per-partition and reduced on the host.
"""

import numpy as np

import concourse.bacc as bacc
import concourse.bass as bass
import concourse.mybir as mybir
import concourse.tile as tile
from concourse import bass_utils
from concourse.masks import make_identity
from concourse.dve_ops import TENSOR_TENSOR_REDUCE

F32 = mybir.dt.float32
I32 = mybir.dt.int32
U32 = mybir.dt.uint32

B, T, D = 4, 512, 128
HDIM, CDIM = 1024, 256
NI = HDIM // CDIM          # 4 iterations
S = 2 * D - 1              # 255 shifts
PADW = 3 * D - 2           # 382 padded width
NCORES = 8
ROWS = (B * T) // NCORES   # 256 rows per core
NT = ROWS // 128           # 2 partition tiles per core
P = 128
IGNORE_OUT = 10000.0

_ALU = mybir.AluOpType
_ACT = mybir.ActivationFunctionType

_NC_CACHE = {}


def _body(tc):
    nc = tc.nc

    xin = nc.dram_tensor("xin", [ROWS, D], F32, kind="ExternalInput").ap()
    yin = nc.dram_tensor("yin", [ROWS, D], F32, kind="ExternalInput").ap()
    w1t = nc.dram_tensor("w1t", [D, HDIM], F32, kind="ExternalInput").ap()
    w2t = nc.dram_tensor("w2t", [P, HDIM // P, D], F32, kind="ExternalInput").ap()
    b1c = nc.dram_tensor("b1c", [P, HDIM // P], F32, kind="ExternalInput").ap()
    b2c = nc.dram_tensor("b2c", [P, 1], F32, kind="ExternalInput").ap()
    cfd = nc.dram_tensor("cfd", [D, P], F32, kind="ExternalInput").ap()
    sfd = nc.dram_tensor("sfd", [D, P], F32, kind="ExternalInput").ap()
    wcd = nc.dram_tensor("wcd", [P, 2 * P], F32, kind="ExternalInput").ap()
    wsd = nc.dram_tensor("wsd", [P, 2 * P], F32, kind="ExternalInput").ap()
    lout = nc.dram_tensor("lsum", [NT, P, NI], F32, kind="ExternalOutput").ap()

    with (
        tc.tile_pool(name="singles", bufs=1) as singles,
        tc.tile_pool(name="dramp", bufs=1, space="DRAM") as dramp,
        tc.tile_pool(name="work", bufs=2) as work,
        tc.tile_pool(name="psum", bufs=1, space="PSUM") as psum,
    ):
        # --- persistent state ------------------------------------------------
        xp = [singles.tile([P, PADW], F32, tag=f"xp{t}", name=f"xp{t}") for t in range(NT)]
        yr = [singles.tile([P, D], F32, tag=f"yr{t}", name=f"yr{t}") for t in range(NT)]
        xap = [singles.tile([P, PADW], F32, tag=f"xap{t}", name=f"xap{t}") for t in range(NT)]
        xpd = [dramp.tile([P, PADW], F32, tag=f"xpd{t}", name=f"xpd{t}") for t in range(NT)]
        xapd = [dramp.tile([P, PADW], F32, tag=f"xapd{t}", name=f"xapd{t}") for t in range(NT)]
        w1s = singles.tile([P, HDIM], F32)
        w2s = singles.tile([P, HDIM // P, D], F32)
        b1s = singles.tile([P, HDIM // P], F32)
        b2s = singles.tile([P, 1], F32)
        cfs = singles.tile([D, P], F32)
        sfs = singles.tile([D, P], F32)
        wcs = singles.tile([P, 2 * P], F32)
        wss = singles.tile([P, 2 * P], F32)
        ident = singles.tile([P, P], F32)
        iota_a = singles.tile([P, 1], U32)   # p*PADW
        iota_e = singles.tile([P, 1], U32)   # p*PADW + (S-1)
        lsum = singles.tile([P, NT * NI], F32)
        zero1 = singles.tile([P, 1], F32)

        for t in range(NT):
            nc.gpsimd.memset(xp[t], 0.0)
            nc.gpsimd.memset(xap[t], 0.0)
            nc.sync.dma_start(out=xp[t][:, D - 1 : D - 1 + D],
                              in_=xin[t * P : (t + 1) * P, :])
            nc.sync.dma_start(out=yr[t], in_=yin[t * P : (t + 1) * P, :])
        nc.sync.dma_start(out=w1s, in_=w1t)
        nc.sync.dma_start(out=w2s, in_=w2t)
        nc.sync.dma_start(out=b1s, in_=b1c)
        nc.sync.dma_start(out=b2s, in_=b2c)
        nc.sync.dma_start(out=cfs, in_=cfd)
        nc.sync.dma_start(out=sfs, in_=sfd)
        nc.sync.dma_start(out=wcs, in_=wcd)
        nc.sync.dma_start(out=wss, in_=wsd)
        make_identity(nc, ident)
        nc.gpsimd.memset(zero1, 0.0)
        nc.gpsimd.iota(iota_a, pattern=[[0, 1]], base=0, channel_multiplier=PADW)
        nc.gpsimd.iota(iota_e, pattern=[[0, 1]], base=S - 1, channel_multiplier=PADW)

        for i in range(NI):
            hblks = (2 * i, 2 * i + 1)

            # --- sliding correlation via 255-pt circular DFT (both tiles) ---
            # num[p,s] = sum_d y[p,d]*xpad[p,s+d] = c[(s+128) mod 255] where
            # c = circ-corr(x,y) at 255 points (exactly linear: 255=2*128-1).
            # The (s+128)%255 remap and the 1/255, x2 Hermitian-fold factors
            # are baked into the host-built inverse matrices wcs/wss.
            xT = work.tile([D, NT * P], F32, tag="xTall")
            yT = work.tile([D, NT * P], F32, tag="yTall")
            for t in range(NT):
                # mirror padded x_res to DRAM for the per-row window gather
                nc.sync.dma_start(out=xpd[t], in_=xp[t])
                tr_ps = psum.tile([P, 2, P], F32, tag=f"trp{t}")
                nc.tensor.transpose(out=tr_ps[:, 0], in_=xp[t][:, D - 1 : D - 1 + D],
                                    identity=ident)
                nc.scalar.activation(xT[:, t * P : (t + 1) * P], tr_ps[:, 0], _ACT.Copy)
                nc.tensor.transpose(out=tr_ps[:, 1], in_=yr[t], identity=ident)
                nc.scalar.activation(yT[:, t * P : (t + 1) * P], tr_ps[:, 1], _ACT.Copy)

            X_ps = psum.tile([P, 2, NT * P], F32, tag="Xps")
            nc.tensor.matmul(X_ps[:, 0], lhsT=cfs, rhs=xT, start=True, stop=True)
            nc.tensor.matmul(X_ps[:, 1], lhsT=sfs, rhs=xT, start=True, stop=True)
            Y_ps = psum.tile([P, 2, NT * P], F32, tag="Yps")
            nc.tensor.matmul(Y_ps[:, 0], lhsT=cfs, rhs=yT, start=True, stop=True)
            nc.tensor.matmul(Y_ps[:, 1], lhsT=sfs, rhs=yT, start=True, stop=True)
            X_s = work.tile([P, 2, NT * P], F32, tag="Xs")
            Y_s = work.tile([P, 2, NT * P], F32, tag="Ys")
            nc.scalar.activation(X_s, X_ps, _ACT.Copy)
            nc.scalar.activation(Y_s, Y_ps, _ACT.Copy)

            # Z = F(x) * conj(F(y)): DVE does Zr, gpsimd does Zi in parallel
            zt1 = work.tile([P, NT * P], F32, tag="zt1")
            zt2 = work.tile([P, NT * P], F32, tag="zt2")
            Zr_s = work.tile([P, NT * P], F32, tag="Zr")
            nc.vector.tensor_tensor(zt1, X_s[:, 0], Y_s[:, 0], op=_ALU.mult)
            nc.vector.tensor_tensor(zt2, X_s[:, 1], Y_s[:, 1], op=_ALU.mult)
            nc.vector.tensor_tensor(Zr_s, zt1, zt2, op=_ALU.add)
            zt3 = work.tile([P, NT * P], F32, tag="zt3")
            zt4 = work.tile([P, NT * P], F32, tag="zt4")
            Zi_s = work.tile([P, NT * P], F32, tag="Zi")
            nc.gpsimd.tensor_tensor(zt3, X_s[:, 1], Y_s[:, 0], op=_ALU.mult)
            nc.gpsimd.tensor_tensor(zt4, X_s[:, 0], Y_s[:, 1], op=_ALU.mult)
            nc.gpsimd.tensor_tensor(Zi_s, zt3, zt4, op=_ALU.subtract)

            # inverse: num_T[s-block] = WC_b^T Zr + WS_b^T Zi  (PSUM accum)
            # (reuses the Xps bank — X_ps is dead once Zr/Zi are formed)
            nT_ps = psum.tile([P, 2, NT * P], F32, tag="Xps")
            nc.tensor.matmul(nT_ps[:, 0], lhsT=wcs[:, 0:P], rhs=Zr_s,
                             start=True, stop=False)
            nc.tensor.matmul(nT_ps[:, 0], lhsT=wss[:, 0:P], rhs=Zi_s,
                             start=False, stop=True)
            nc.tensor.matmul(nT_ps[:, 1], lhsT=wcs[:, P : 2 * P], rhs=Zr_s,
                             start=True, stop=False)
            nc.tensor.matmul(nT_ps[:, 1], lhsT=wss[:, P : 2 * P], rhs=Zi_s,
                             start=False, stop=True)
            nT_s = work.tile([P, 2, NT * P], F32, tag="nTs")
            nc.scalar.activation(nT_s, nT_ps, _ACT.Copy)
            nrm_ps = psum.tile([P, NT, 2 * P], F32, tag="nrm")

            for t in range(NT):
                # --- window norms via two cancellation-free prefix scans ----
                # left-edge windows (s<=127) overlap x[0..s]: prefix sums;
                # right-edge windows overlap x[s-127..127]: suffix sums from a
                # scan over the reversed x^2. The 1e-30 seed guards 0/0.
                x2m = work.tile([P, D], F32, tag="x2m")
                nc.scalar.activation(x2m, xp[t][:, D - 1 : D - 1 + D], _ACT.Square)
                ss2 = work.tile([P, S], F32, tag="ss2")
                nc.vector.tensor_tensor_scan(
                    out=ss2[:, 0:D], data0=x2m, data1=x2m,
                    initial=1e-30, op0=_ALU.add, op1=_ALU.bypass)
                # right-edge windows in one pass: reversed-read scan of x^2
                # with reversed write lands suffix[j] at column 127+j
                nc.vector.tensor_tensor_scan(
                    out=ss2[:, S - 1 : D - 1 : -1],
                    data0=x2m[:, D - 1 : 0 : -1], data1=x2m[:, D - 1 : 0 : -1],
                    initial=1e-30, op0=_ALU.add, op1=_ALU.bypass)

                # --- transpose num back to row-major [r, s] -----------------
                num_ps = nrm_ps[:, t]
                nc.tensor.transpose(out=num_ps[:, 0:P],
                                    in_=nT_s[:, 0, t * P : (t + 1) * P],
                                    identity=ident)
                nc.tensor.transpose(out=num_ps[:, P : 2 * P],
                                    in_=nT_s[:, 1, t * P : (t + 1) * P],
                                    identity=ident)

                # --- score num/sqrt(ss) (||y|| factor dropped) + argmax -----
                sq = work.tile([P, S], F32, tag="sq")
                nc.scalar.activation(sq, ss2, _ACT.Sqrt)
                rec = work.tile([P, S], F32, tag="rec")
                nc.vector.reciprocal_approx_fast(rec, sq)
                simv = work.tile([P, S], F32, tag="simv")
                nc.vector.tensor_tensor(simv, num_ps[:, 0:S], rec, op=_ALU.mult)
                maxv = work.tile([P, 8], F32, tag="maxv")
                idx8 = work.tile([P, 8], U32, tag="idx8")
                nc.vector.max_with_indices(maxv, idx8, simv)

                # --- gather best window: x_aug[p,:] = xp[p, idx[p]:idx[p]+128]
                offa = work.tile([P, 1], U32, tag="offa")
                nc.gpsimd.tensor_tensor(offa, iota_a, idx8[:, 0:1], op=_ALU.add)
                xaug = work.tile([P, D], F32, tag="xaug")
                nc.gpsimd.indirect_dma_start(
                    out=xaug, out_offset=None,
                    in_=xpd[t][:].rearrange("p (w o) -> (p w) o", o=1),
                    in_offset=bass.IndirectOffsetOnAxis(ap=offa, axis=0))

                # --- attention: x_attn = x_aug * softmax(x_aug*y) -----------
                tmul = work.tile([P, D], F32, tag="tmul")
                mval = work.tile([P, 1], F32, tag="mval")
                nc.vector.tensor_mul(tmul, xaug, yr[t])
                nc.vector.reduce_max(out=mval, in_=tmul, axis=mybir.AxisListType.X)
                negm = work.tile([P, 1], F32, tag="negm")
                nc.gpsimd.tensor_tensor(negm, zero1, mval, op=_ALU.subtract)
                e1 = work.tile([P, D], F32, tag="e1")
                se = work.tile([P, 1], F32, tag="se")
                nc.scalar.activation(e1, tmul, _ACT.Exp, bias=negm[:, 0:1],
                                     scale=1.0, accum_out=se)
                lnse = work.tile([P, 1], F32, tag="lnse")
                nc.scalar.activation(lnse, se, _ACT.Ln)
                bias2 = work.tile([P, 1], F32, tag="bias2")
                nc.gpsimd.tensor_tensor(bias2, negm, lnse, op=_ALU.subtract)
                e2 = work.tile([P, D], F32, tag="e2")
                nc.scalar.activation(e2, tmul, _ACT.Exp, bias=bias2[:, 0:1])
                # x_attn written straight into the padded reverse-shift buffer
                nc.vector.tensor_tensor(
                    xap[t][:, D - 1 : D - 1 + D], xaug, e2, op=_ALU.mult)
                nc.sync.dma_start(out=xapd[t], in_=xap[t])

                # --- reverse shift: x_ele[p,j] = xap[p, 254-idx[p]+j] -------
                offe = work.tile([P, 1], U32, tag="offe")
                nc.gpsimd.tensor_tensor(offe, iota_e, idx8[:, 0:1], op=_ALU.subtract)
                xele = work.tile([P, D], F32, tag="xele")
                nc.gpsimd.indirect_dma_start(
                    out=xele, out_offset=None,
                    in_=xapd[t][:].rearrange("p (w o) -> (p w) o", o=1),
                    in_offset=bass.IndirectOffsetOnAxis(ap=offe, axis=0))
                # x_res -= x_ele (middle of the padded buffer, for next iter)
                nc.gpsimd.tensor_tensor(
                    xp[t][:, D - 1 : D - 1 + D],
                    xp[t][:, D - 1 : D - 1 + D], xele, op=_ALU.subtract)

                # --- masked 2-layer MLP in transposed layout ----------------
                mlpa_ps = psum.tile([P, 2, P], F32, tag="mlpa")
                xT_ps = mlpa_ps[:, 0]
                nc.tensor.transpose(out=xT_ps, in_=xap[t][:, D - 1 : D - 1 + D],
                                    identity=ident)
                xTm = work.tile([P, P], F32, tag="xT")
                nc.scalar.activation(xTm, xT_ps, _ACT.Copy)
                y_ps = psum.tile([P, P], F32, tag="y_ps")
                hps = psum.tile([P, 2, P], F32, tag="hps")
                for j, hb in enumerate(hblks):
                    h_ps = hps[:, j]
                    nc.tensor.matmul(h_ps, lhsT=w1s[:, hb * P : (hb + 1) * P],
                                     rhs=xTm, start=True, stop=True)
                    hT = work.tile([P, P], F32, tag=f"hT{j}")
                    nc.scalar.activation(hT, h_ps, _ACT.Identity,
                                         bias=b1s[:, hb : hb + 1])
                    nc.tensor.matmul(y_ps, lhsT=w2s[:, hb, :], rhs=hT,
                                     start=(j == 0), stop=(j == 1))
                yT = work.tile([P, P], F32, tag="yT")
                nc.scalar.activation(yT, y_ps, _ACT.Identity, bias=b2s[:, 0:1])
                ye_ps = mlpa_ps[:, 1]
                nc.tensor.transpose(out=ye_ps, in_=yT, identity=ident)

                # --- residual update + loss: sq = (y_ele-y_res)^2 = y_new^2 -
                nc.vector.tensor_tensor(yr[t], yr[t], ye_ps, op=_ALU.subtract)
                slot = t * NI + i
                prev = 0.0 if i == 0 else lsum[:, slot - 1 : slot]
                prod2 = work.tile([P, D], F32, tag="prod2")
                nc.vector._custom_dve(
                    TENSOR_TENSOR_REDUCE,
                    out=prod2, in0=yr[t], in1=yr[t], s0=prev, s1=1.0,
                    accum_out=lsum[:, slot : slot + 1])

        for t in range(NT):
            nc.sync.dma_start(out=lout[t],
                              in_=lsum[:, t * NI : (t + 1) * NI])


def build_nc():
    if "nc" in _NC_CACHE:
        return _NC_CACHE["nc"]
    nc = bacc.Bacc("TRN2", target_bir_lowering=False, debug=False,
                   enable_asserts=True, num_devices=NCORES)
    with tile.TileContext(nc) as tc:
        _body(tc)
    nc.compile()
    _NC_CACHE["nc"] = nc
    return nc


def make_in_maps(x, y, w1, b1, w2, b2):
    x = np.ascontiguousarray(np.asarray(x, np.float32)).reshape(B * T, D)
    y = np.ascontiguousarray(np.asarray(y, np.float32)).reshape(B * T, D)
    w1 = np.asarray(w1, np.float32)
    b1 = np.asarray(b1, np.float32)
    w2 = np.asarray(w2, np.float32)
    b2 = np.asarray(b2, np.float32)
    w1t = np.ascontiguousarray(w1.T)                      # (128, 1024)
    w2t = np.ascontiguousarray(                            # (128, 8, 128)
        w2.T.reshape(HDIM // P, P, D).transpose(1, 0, 2))
    b1c = np.ascontiguousarray(b1.reshape(HDIM // P, P).T)  # (128, 8)
    b2c = np.ascontiguousarray(b2.reshape(D, 1))             # (128, 1)
    cfd, sfd, wcd, wsd = _dft_mats()
    maps = []
    for c in range(NCORES):
        maps.append({
            "xin": np.ascontiguousarray(x[c * ROWS : (c + 1) * ROWS]),
            "yin": np.ascontiguousarray(y[c * ROWS : (c + 1) * ROWS]),
            "w1t": w1t, "w2t": w2t, "b1c": b1c, "b2c": b2c,
            "cfd": cfd, "sfd": sfd, "wcd": wcd, "wsd": wsd,
        })
    return maps


def _dft_mats():
    """Real 255-point DFT matrices for the sliding correlation.

    Forward (freqs k=0..127; bins 128..254 are the Hermitian mirror):
      Xr = cfd.T @ x, Xi = sfd.T @ x with cfd[d,k]=cos(thkd), sfd=-sin.
    Inverse, with the 1/255 norm, the x2 Hermitian fold (k>0), and the
    s -> (s+128) mod 255 lag remap baked in; column 255 is zero so the
    transposed-back num tile carries a harmless 0 in its junk column:
      num_T = wcd[:, blk].T @ Zr + wsd[:, blk].T @ Zi.
    """
    th = 2.0 * np.pi / S
    k = np.arange(P, dtype=np.float64)
    dd = np.arange(D, dtype=np.float64)
    cfd = np.cos(th * np.outer(dd, k)).astype(np.float32)
    sfd = (-np.sin(th * np.outer(dd, k))).astype(np.float32)
    u = (np.arange(S, dtype=np.int64) + D) % S
    alpha = np.full(P, 2.0 / S, dtype=np.float64)
    alpha[0] = 1.0 / S
    wcd = np.zeros((P, 2 * P), np.float32)
    wsd = np.zeros((P, 2 * P), np.float32)
    wcd[:, :S] = (alpha[:, None] * np.cos(th * np.outer(k, u))).astype(np.float32)
    wsd[:, :S] = (-alpha[:, None] * np.sin(th * np.outer(k, u))).astype(np.float32)
    return (np.ascontiguousarray(cfd), np.ascontiguousarray(sfd),
            np.ascontiguousarray(wcd), np.ascontiguousarray(wsd))


def finalize(lsums, y):
    """lsums: list of per-core (NT, P, NI) partial sums of squares."""
    denom = np.float64((np.asarray(y) != IGNORE_OUT).sum())
    total = np.float64(0.0)
    for ls in lsums:
        # slot NI-1 of each (t) chain holds that tile's total over iterations
        total += np.float64(ls[:, :, NI - 1].sum(dtype=np.float64))
    return np.float32(total / denom / NI)


def kernel(x, y, w1, b1, w2, b2):
    nc = build_nc()
    in_maps = make_in_maps(x, y, w1, b1, w2, b2)
    res = bass_utils.run_bass_kernel_spmd(nc, in_maps, core_ids=list(range(NCORES)))
    lsums = [res.results[c]["lsum"] for c in range(NCORES)]
    return finalize(lsums, y)



# revision 28
# speedup vs baseline: 1.2241x; 1.2241x over previous
"""Trainium2 Bass kernel for nn_Net_41223096107028.

Computes the 4-iteration argaug/attention/masked-MLP loss of reference.py
on 8 NeuronCores, data-parallel over the 2048 (b,t) rows (256 rows/core,
2 partition-tiles of 128).

Per iteration:
  - sliding correlation num[p,s] = <y_res[p], window_s(x_res[p])> via an
    exact 255-point circular DFT on the PE array: num = IDFT(F(x)conj(F(y)))
    with fixed real DFT matrices (255 = 2*128-1, so circular == linear
    correlation exactly; per-row correlations can't be a direct matmul, but
    the DFT factorization shares its matrices across rows). 8 fp32 matmuls
    per iteration over all 256 rows replaces 2040 truncated-window DVE
    reduce ops.
  - window norms via two cancellation-free DVE prefix scans of x^2,
  - argmax over num/sqrt(ss) (||y|| > 0 is a common positive factor and is
    dropped; reciprocal via the 1-instruction approx op),
  - per-row window gathers via indirect DMA on a DRAM mirror (per-partition
    offsets; gpsimd indirect_copy shares indices across 16-partition groups
    so it cannot do per-row shifts),
  - softmax folded into a second ACT Exp pass with bias = -max - ln(sum),
  - the 2-layer channel-masked MLP as 4 PE matmuls in transposed layout
    (only the active 256-channel slice is computed),
  - loss via ||y_res_new||^2 (y_ele - y_res = -y_res_new), accumulated
    per-partition and reduced on the host.
"""

import numpy as np

import concourse.bacc as bacc
import concourse.bass as bass
import concourse.mybir as mybir
import concourse.tile as tile
from concourse import bass_utils
from concourse.masks import make_identity
from concourse.dve_ops import TENSOR_TENSOR_REDUCE

F32 = mybir.dt.float32
I32 = mybir.dt.int32
U32 = mybir.dt.uint32

B, T, D = 4, 512, 128
HDIM, CDIM = 1024, 256
NI = HDIM // CDIM          # 4 iterations
S = 2 * D - 1              # 255 shifts
PADW = 3 * D - 2           # 382 padded width
NCORES = 8
ROWS = (B * T) // NCORES   # 256 rows per core
NT = ROWS // 128           # 2 partition tiles per core
P = 128
IGNORE_OUT = 10000.0

_ALU = mybir.AluOpType
_ACT = mybir.ActivationFunctionType
# float32r: the PE reads FP22-truncated operands at 1 cycle/row (vs 4 for
# fp32) when the moving dim is >= 256. Every producer writing a tile that a
# f32r matmul consumes must itself be typed f32r (BIR verifier), so the
# affected tiles/DRAM tensors are declared F32R outright. Loss impact
# measured at 3e-6 relative (vs the 2e-2 gate).
F32R = mybir.dt.float32r

_NC_CACHE = {}


def _body(tc):
    nc = tc.nc

    xin = nc.dram_tensor("xin", [ROWS, D], F32, kind="ExternalInput").ap()
    yin = nc.dram_tensor("yin", [ROWS, D], F32, kind="ExternalInput").ap()
    w1t = nc.dram_tensor("w1t", [D, HDIM], F32R, kind="ExternalInput").ap()
    w2t = nc.dram_tensor("w2t", [P, HDIM // P, D], F32R, kind="ExternalInput").ap()
    b1c = nc.dram_tensor("b1c", [P, HDIM // P], F32, kind="ExternalInput").ap()
    b2c = nc.dram_tensor("b2c", [P, 1], F32, kind="ExternalInput").ap()
    cfd = nc.dram_tensor("cfd", [D, P], F32R, kind="ExternalInput").ap()
    sfd = nc.dram_tensor("sfd", [D, P], F32R, kind="ExternalInput").ap()
    wcd = nc.dram_tensor("wcd", [P, 2 * P], F32R, kind="ExternalInput").ap()
    wsd = nc.dram_tensor("wsd", [P, 2 * P], F32R, kind="ExternalInput").ap()
    lout = nc.dram_tensor("lsum", [NT, P, NI], F32, kind="ExternalOutput").ap()

    with (
        tc.tile_pool(name="singles", bufs=1) as singles,
        tc.tile_pool(name="dramp", bufs=1, space="DRAM") as dramp,
        tc.tile_pool(name="work", bufs=2) as work,
        tc.tile_pool(name="psum", bufs=1, space="PSUM") as psum,
    ):
        # --- persistent state ------------------------------------------------
        xp = [singles.tile([P, PADW], F32, tag=f"xp{t}", name=f"xp{t}") for t in range(NT)]
        yr = [singles.tile([P, D], F32, tag=f"yr{t}", name=f"yr{t}") for t in range(NT)]
        xap = [singles.tile([P, PADW], F32, tag=f"xap{t}", name=f"xap{t}") for t in range(NT)]
        xpd = [dramp.tile([P, PADW], F32, tag=f"xpd{t}", name=f"xpd{t}") for t in range(NT)]
        xapd = [dramp.tile([P, PADW], F32, tag=f"xapd{t}", name=f"xapd{t}") for t in range(NT)]
        w1s = singles.tile([P, HDIM], F32R)
        w2s = singles.tile([P, HDIM // P, D], F32R)
        b1s = singles.tile([P, HDIM // P], F32)
        b2s = singles.tile([P, 1], F32)
        cfs = singles.tile([D, P], F32R)
        sfs = singles.tile([D, P], F32R)
        wcs = singles.tile([P, 2 * P], F32R)
        wss = singles.tile([P, 2 * P], F32R)
        ident = singles.tile([P, P], F32)
        iota_a = singles.tile([P, 1], U32)   # p*PADW
        iota_e = singles.tile([P, 1], U32)   # p*PADW + (S-1)
        lsum = singles.tile([P, NT * NI], F32)
        zero1 = singles.tile([P, 1], F32)

        yTs = singles.tile([D, NT * P], F32R)   # persistent transposed y_res
        for t in range(NT):
            nc.gpsimd.memset(xp[t], 0.0)
            nc.gpsimd.memset(xap[t], 0.0)
            nc.sync.dma_start(out=xp[t][:, D - 1 : D - 1 + D],
                              in_=xin[t * P : (t + 1) * P, :])
            nc.sync.dma_start(out=yr[t], in_=yin[t * P : (t + 1) * P, :])
        nc.sync.dma_start(out=w1s, in_=w1t)
        nc.sync.dma_start(out=w2s, in_=w2t)
        nc.sync.dma_start(out=b1s, in_=b1c)
        nc.sync.dma_start(out=b2s, in_=b2c)
        nc.sync.dma_start(out=cfs, in_=cfd)
        nc.sync.dma_start(out=sfs, in_=sfd)
        nc.sync.dma_start(out=wcs, in_=wcd)
        nc.sync.dma_start(out=wss, in_=wsd)
        make_identity(nc, ident)
        nc.gpsimd.memset(zero1, 0.0)
        nc.gpsimd.iota(iota_a, pattern=[[0, 1]], base=0, channel_multiplier=PADW)
        nc.gpsimd.iota(iota_e, pattern=[[0, 1]], base=S - 1, channel_multiplier=PADW)
        for t in range(NT):
            tr0_ps = psum.tile([P, 2, P], F32, tag=f"trp{t}")
            nc.tensor.transpose(out=tr0_ps[:, 0], in_=yr[t], identity=ident)
            nc.scalar.activation(yTs[:, t * P : (t + 1) * P], tr0_ps[:, 0],
                                 _ACT.Copy)

        for i in range(NI):
            hblks = (2 * i, 2 * i + 1)

            # --- sliding correlation via 255-pt circular DFT (both tiles) ---
            # num[p,s] = sum_d y[p,d]*xpad[p,s+d] = c[(s+128) mod 255] where
            # c = circ-corr(x,y) at 255 points (exactly linear: 255=2*128-1).
            # The (s+128)%255 remap and the 1/255, x2 Hermitian-fold factors
            # are baked into the host-built inverse matrices wcs/wss.
            xT = work.tile([D, NT * P], F32R, tag="xTall")
            for t in range(NT):
                # mirror padded x_res to DRAM for the per-row window gather
                nc.sync.dma_start(out=xpd[t], in_=xp[t])
                tr_ps = psum.tile([P, 2, P], F32, tag=f"trp{t}")
                nc.tensor.transpose(out=tr_ps[:, 0], in_=xp[t][:, D - 1 : D - 1 + D],
                                    identity=ident)
                nc.scalar.activation(xT[:, t * P : (t + 1) * P], tr_ps[:, 0], _ACT.Copy)

            X_ps = psum.tile([P, 2, NT * P], F32, tag="Xps")
            nc.tensor.matmul(X_ps[:, 0], lhsT=cfs, rhs=xT, start=True, stop=True)
            nc.tensor.matmul(X_ps[:, 1], lhsT=sfs, rhs=xT, start=True, stop=True)
            Y_ps = psum.tile([P, 2, NT * P], F32, tag="Yps")
            nc.tensor.matmul(Y_ps[:, 0], lhsT=cfs, rhs=yTs, start=True, stop=True)
            nc.tensor.matmul(Y_ps[:, 1], lhsT=sfs, rhs=yTs, start=True, stop=True)
            X_s = work.tile([P, 2, NT * P], F32, tag="Xs")
            Y_s = work.tile([P, 2, NT * P], F32, tag="Ys")
            nc.scalar.activation(X_s, X_ps, _ACT.Copy)
            nc.scalar.activation(Y_s, Y_ps, _ACT.Copy)

            # Z = F(x) * conj(F(y)): DVE does Zr, gpsimd does Zi in parallel
            zt1 = work.tile([P, NT * P], F32, tag="zt1")
            zt2 = work.tile([P, NT * P], F32, tag="zt2")
            Zr_s = work.tile([P, NT * P], F32R, tag="Zr")
            nc.vector.tensor_tensor(zt1, X_s[:, 0], Y_s[:, 0], op=_ALU.mult)
            nc.vector.tensor_tensor(zt2, X_s[:, 1], Y_s[:, 1], op=_ALU.mult)
            nc.vector.tensor_tensor(Zr_s, zt1, zt2, op=_ALU.add)
            zt3 = work.tile([P, NT * P], F32, tag="zt3")
            zt4 = work.tile([P, NT * P], F32, tag="zt4")
            Zi_s = work.tile([P, NT * P], F32R, tag="Zi")
            nc.gpsimd.tensor_tensor(zt3, X_s[:, 1], Y_s[:, 0], op=_ALU.mult)
            nc.gpsimd.tensor_tensor(zt4, X_s[:, 0], Y_s[:, 1], op=_ALU.mult)
            nc.gpsimd.tensor_tensor(Zi_s, zt3, zt4, op=_ALU.subtract)

            # inverse: num_T[s-block] = WC_b^T Zr + WS_b^T Zi  (PSUM accum)
            # (reuses the Xps bank — X_ps is dead once Zr/Zi are formed)
            nT_ps = psum.tile([P, 2, NT * P], F32, tag="Xps")
            nc.tensor.matmul(nT_ps[:, 0], lhsT=wcs[:, 0:P], rhs=Zr_s,
                             start=True, stop=False)
            nc.tensor.matmul(nT_ps[:, 0], lhsT=wss[:, 0:P], rhs=Zi_s,
                             start=False, stop=True)
            nc.tensor.matmul(nT_ps[:, 1], lhsT=wcs[:, P : 2 * P], rhs=Zr_s,
                             start=True, stop=False)
            nc.tensor.matmul(nT_ps[:, 1], lhsT=wss[:, P : 2 * P], rhs=Zi_s,
                             start=False, stop=True)
            nT_s = work.tile([P, 2, NT * P], F32, tag="nTs")
            nc.scalar.activation(nT_s, nT_ps, _ACT.Copy)
            nrm_ps = psum.tile([P, NT, 2 * P], F32, tag="nrm")
            mlpa_ps = psum.tile([P, NT, P], F32, tag="mlpa")
            xTa = work.tile([P, NT * P], F32R, tag="xTa")

            for t in range(NT):
                # --- window norms via two cancellation-free prefix scans ----
                # left-edge windows (s<=127) overlap x[0..s]: prefix sums;
                # right-edge windows overlap x[s-127..127]: suffix sums from a
                # scan over the reversed x^2. The 1e-30 seed guards 0/0.
                x2m = work.tile([P, D], F32, tag="x2m")
                nc.scalar.activation(x2m, xp[t][:, D - 1 : D - 1 + D], _ACT.Square)
                ss2 = work.tile([P, S], F32, tag="ss2")
                nc.vector.tensor_tensor_scan(
                    out=ss2[:, 0:D], data0=x2m, data1=x2m,
                    initial=1e-30, op0=_ALU.add, op1=_ALU.bypass)
                # right-edge windows in one pass: reversed-read scan of x^2
                # with reversed write lands suffix[j] at column 127+j
                nc.vector.tensor_tensor_scan(
                    out=ss2[:, S - 1 : D - 1 : -1],
                    data0=x2m[:, D - 1 : 0 : -1], data1=x2m[:, D - 1 : 0 : -1],
                    initial=1e-30, op0=_ALU.add, op1=_ALU.bypass)

                # --- transpose num back to row-major [r, s] -----------------
                num_ps = nrm_ps[:, t]
                nc.tensor.transpose(out=num_ps[:, 0:P],
                                    in_=nT_s[:, 0, t * P : (t + 1) * P],
                                    identity=ident)
                nc.tensor.transpose(out=num_ps[:, P : 2 * P],
                                    in_=nT_s[:, 1, t * P : (t + 1) * P],
                                    identity=ident)

                # --- score num*|num|/ss (monotone in num/sqrt(ss); avoids
                # ACT Sqrt, whose sel=1 table swap costs 2x1.28us per iter) --
                num_s = work.tile([P, 2 * P], F32, tag="num_s")
                nc.scalar.activation(num_s, num_ps, _ACT.Copy)
                nabs = work.tile([P, S], F32, tag="nabs")
                nc.vector.tensor_scalar(
                    out=nabs.bitcast(U32), in0=num_s[:, 0:S].bitcast(U32),
                    scalar1=0x7FFFFFFF, scalar2=None, op0=_ALU.bitwise_and)
                nsq = work.tile([P, S], F32, tag="nsq")
                nc.gpsimd.tensor_tensor(nsq, num_s[:, 0:S], nabs, op=_ALU.mult)
                rec = work.tile([P, S], F32, tag="rec")
                nc.vector.reciprocal_approx_fast(rec, ss2)
                simv = work.tile([P, S], F32, tag="simv")
                nc.vector.tensor_tensor(simv, nsq, rec, op=_ALU.mult)
                maxv = work.tile([P, 8], F32, tag="maxv")
                idx8 = work.tile([P, 8], U32, tag="idx8")
                nc.vector.max_with_indices(maxv, idx8, simv)

                # --- gather best window: x_aug[p,:] = xp[p, idx[p]:idx[p]+128]
                offa = work.tile([P, 1], U32, tag="offa")
                nc.gpsimd.tensor_tensor(offa, iota_a, idx8[:, 0:1], op=_ALU.add)
                xaug = work.tile([P, D], F32, tag="xaug")
                nc.gpsimd.indirect_dma_start(
                    out=xaug, out_offset=None,
                    in_=xpd[t][:].rearrange("p (w o) -> (p w) o", o=1),
                    in_offset=bass.IndirectOffsetOnAxis(ap=offa, axis=0))

                # --- attention: x_attn = x_aug * softmax(x_aug*y) -----------
                # softmax as e1/sum(e1): one Exp pass + approx reciprocal
                # (the Ln/2nd-Exp variant costs 2 ACT table swaps per iter)
                tmul = work.tile([P, D], F32, tag="tmul")
                mval = work.tile([P, 1], F32, tag="mval")
                nc.vector.tensor_mul(tmul, xaug, yr[t])
                nc.vector.reduce_max(out=mval, in_=tmul, axis=mybir.AxisListType.X)
                negm = work.tile([P, 1], F32, tag="negm")
                nc.gpsimd.tensor_tensor(negm, zero1, mval, op=_ALU.subtract)
                e1 = work.tile([P, D], F32, tag="e1")
                se = work.tile([P, 1], F32, tag="se")
                nc.scalar.activation(e1, tmul, _ACT.Exp, bias=negm[:, 0:1],
                                     scale=1.0, accum_out=se)
                recse = work.tile([P, 1], F32, tag="recse")
                nc.vector.reciprocal_approx_fast(recse, se)
                xae = work.tile([P, D], F32, tag="xae")
                nc.gpsimd.tensor_tensor(xae, xaug, e1, op=_ALU.mult)
                # x_attn written straight into the padded reverse-shift buffer
                nc.vector.tensor_scalar_mul(
                    xap[t][:, D - 1 : D - 1 + D], xae, recse[:, 0:1])
                nc.sync.dma_start(out=xapd[t], in_=xap[t])

                # --- reverse shift: x_ele[p,j] = xap[p, 254-idx[p]+j] -------
                offe = work.tile([P, 1], U32, tag="offe")
                nc.gpsimd.tensor_tensor(offe, iota_e, idx8[:, 0:1], op=_ALU.subtract)
                xele = work.tile([P, D], F32, tag="xele")
                nc.gpsimd.indirect_dma_start(
                    out=xele, out_offset=None,
                    in_=xapd[t][:].rearrange("p (w o) -> (p w) o", o=1),
                    in_offset=bass.IndirectOffsetOnAxis(ap=offe, axis=0))
                # x_res -= x_ele (middle of the padded buffer, for next iter)
                nc.gpsimd.tensor_tensor(
                    xp[t][:, D - 1 : D - 1 + D],
                    xp[t][:, D - 1 : D - 1 + D], xele, op=_ALU.subtract)

                # transpose x_attn for the (tile-merged) MLP
                nc.tensor.transpose(out=mlpa_ps[:, t],
                                    in_=xap[t][:, D - 1 : D - 1 + D],
                                    identity=ident)
                nc.scalar.activation(xTa[:, t * P : (t + 1) * P], mlpa_ps[:, t],
                                     _ACT.Copy)

            # --- masked 2-layer MLP, both tiles at once (halves LDWEIGHTS) --
            hps = psum.tile([P, 2, NT * P], F32, tag="hps")
            hT = work.tile([P, 2, NT * P], F32R, tag="hTa")
            y_ps = psum.tile([P, NT * P], F32, tag="y_ps")
            for j, hb in enumerate(hblks):
                nc.tensor.matmul(hps[:, j], lhsT=w1s[:, hb * P : (hb + 1) * P],
                                 rhs=xTa, start=True, stop=True)
                nc.scalar.activation(hT[:, j], hps[:, j], _ACT.Identity,
                                     bias=b1s[:, hb : hb + 1])
                nc.tensor.matmul(y_ps, lhsT=w2s[:, hb, :], rhs=hT[:, j],
                                 start=(j == 0), stop=(j == 1))
            yTv = work.tile([P, NT * P], F32, tag="yTv")
            nc.scalar.activation(yTv, y_ps, _ACT.Identity, bias=b2s[:, 0:1])
            # transposed y_res state update (feeds next iter's Y DFT directly)
            nc.vector.tensor_tensor(yTs, yTs, yTv, op=_ALU.subtract)

            for t in range(NT):
                # --- row-major residual update + loss: (y_ele-y_res)^2 ------
                tr2_ps = psum.tile([P, 2, P], F32, tag=f"trp{t}")
                nc.tensor.transpose(out=tr2_ps[:, 1],
                                    in_=yTv[:, t * P : (t + 1) * P],
                                    identity=ident)
                nc.vector.tensor_tensor(yr[t], yr[t], tr2_ps[:, 1],
                                        op=_ALU.subtract)
                slot = t * NI + i
                prev = 0.0 if i == 0 else lsum[:, slot - 1 : slot]
                prod2 = work.tile([P, D], F32, tag="prod2")
                nc.vector._custom_dve(
                    TENSOR_TENSOR_REDUCE,
                    out=prod2, in0=yr[t], in1=yr[t], s0=prev, s1=1.0,
                    accum_out=lsum[:, slot : slot + 1])

        for t in range(NT):
            nc.sync.dma_start(out=lout[t],
                              in_=lsum[:, t * NI : (t + 1) * NI])


def build_nc():
    if "nc" in _NC_CACHE:
        return _NC_CACHE["nc"]
    nc = bacc.Bacc("TRN2", target_bir_lowering=False, debug=False,
                   enable_asserts=True, num_devices=NCORES)
    with tile.TileContext(nc) as tc:
        _body(tc)
    nc.compile()
    _NC_CACHE["nc"] = nc
    return nc


def make_in_maps(x, y, w1, b1, w2, b2):
    x = np.ascontiguousarray(np.asarray(x, np.float32)).reshape(B * T, D)
    y = np.ascontiguousarray(np.asarray(y, np.float32)).reshape(B * T, D)
    w1 = np.asarray(w1, np.float32)
    b1 = np.asarray(b1, np.float32)
    w2 = np.asarray(w2, np.float32)
    b2 = np.asarray(b2, np.float32)
    w1t = np.ascontiguousarray(w1.T)                      # (128, 1024)
    w2t = np.ascontiguousarray(                            # (128, 8, 128)
        w2.T.reshape(HDIM // P, P, D).transpose(1, 0, 2))
    b1c = np.ascontiguousarray(b1.reshape(HDIM // P, P).T)  # (128, 8)
    b2c = np.ascontiguousarray(b2.reshape(D, 1))             # (128, 1)
    cfd, sfd, wcd, wsd = _dft_mats()
    maps = []
    for c in range(NCORES):
        maps.append({
            "xin": np.ascontiguousarray(x[c * ROWS : (c + 1) * ROWS]),
            "yin": np.ascontiguousarray(y[c * ROWS : (c + 1) * ROWS]),
            "w1t": w1t, "w2t": w2t, "b1c": b1c, "b2c": b2c,
            "cfd": cfd, "sfd": sfd, "wcd": wcd, "wsd": wsd,
        })
    return maps


def _dft_mats():
    """Real 255-point DFT matrices for the sliding correlation.

    Forward (freqs k=0..127; bins 128..254 are the Hermitian mirror):
      Xr = cfd.T @ x, Xi = sfd.T @ x with cfd[d,k]=cos(thkd), sfd=-sin.
    Inverse, with the 1/255 norm, the x2 Hermitian fold (k>0), and the
    s -> (s+128) mod 255 lag remap baked in; column 255 is zero so the
    transposed-back num tile carries a harmless 0 in its junk column:
      num_T = wcd[:, blk].T @ Zr + wsd[:, blk].T @ Zi.
    """
    th = 2.0 * np.pi / S
    k = np.arange(P, dtype=np.float64)
    dd = np.arange(D, dtype=np.float64)
    cfd = np.cos(th * np.outer(dd, k)).astype(np.float32)
    sfd = (-np.sin(th * np.outer(dd, k))).astype(np.float32)
    u = (np.arange(S, dtype=np.int64) + D) % S
    alpha = np.full(P, 2.0 / S, dtype=np.float64)
    alpha[0] = 1.0 / S
    wcd = np.zeros((P, 2 * P), np.float32)
    wsd = np.zeros((P, 2 * P), np.float32)
    wcd[:, :S] = (alpha[:, None] * np.cos(th * np.outer(k, u))).astype(np.float32)
    wsd[:, :S] = (-alpha[:, None] * np.sin(th * np.outer(k, u))).astype(np.float32)
    return (np.ascontiguousarray(cfd), np.ascontiguousarray(sfd),
            np.ascontiguousarray(wcd), np.ascontiguousarray(wsd))


def finalize(lsums, y):
    """lsums: list of per-core (NT, P, NI) partial sums of squares."""
    denom = np.float64((np.asarray(y) != IGNORE_OUT).sum())
    total = np.float64(0.0)
    for ls in lsums:
        # slot NI-1 of each (t) chain holds that tile's total over iterations
        total += np.float64(ls[:, :, NI - 1].sum(dtype=np.float64))
    return np.float32(total / denom / NI)


def kernel(x, y, w1, b1, w2, b2):
    nc = build_nc()
    in_maps = make_in_maps(x, y, w1, b1, w2, b2)
    res = bass_utils.run_bass_kernel_spmd(nc, in_maps, core_ids=list(range(NCORES)))
    lsums = [res.results[c]["lsum"] for c in range(NCORES)]
    return finalize(lsums, y)

